# revision 15
# baseline (speedup 1.0000x reference)
"""Trainium2 Bass kernel for GQA attention (B=2, S=2048, D=2048, H=32, KVH=8).

Sharding: 8 cores = 2 batches x 4 head-groups. Each core handles one batch and
8 q-heads / 2 kv-heads: wq/wk/wv column-parallel, wo row-parallel; the partial
wo products are summed on the host.

Host-side prep (pure layout, no math): inputs are sharded, head-permuted and
pre-transposed so every matmul operand DMAs straight into its [K-on-partition]
layout; cos/sin of the rope angles are also computed host-side (the ScalarE Sin
LUT only covers [-pi, pi]).

Per-core kernel (all matmuls float32r):
  - q/k/v projections computed with s on partitions ([s,o] layout) from the
    pre-transposed xT/wqT/wkvT, RoPE applied with strided DVE ops, then q/k
    transposed on PE to [o,s] for attention.
  - scores are computed transposed: scT[k,q] = kT.T @ qT per head; exp on ACT;
    causal handled by skipping fully-masked k-tiles + affine_select on diagonal.
  - PV: lhsT = [v_head | ones] (M=65) so the softmax denominator accumulates in
    PSUM row 64 alongside the output.  attnT = outT * Zinv with Zinv = exp(-ln Z)
    (two small ACT ops) broadcast across partitions via a K=1 PE matmul.
  - final: res[s,d] = sum_p attnT_p.T @ woT_p, accumulated over 4 o-blocks.

Head order within a core is permuted to [0,4,1,5,2,6,3,7] so that each
128-partition block pairs head h (kv0) with h+4 (kv1), letting the K=64 score
matmuls row-pack two heads concurrently on the PE array.
"""

import os
import sys

for _p in ("/opt/trn_rl_repo", "/root/.axon_site/_ro/trn_rl_repo"):
    if os.path.isdir(_p) and _p not in sys.path:
        sys.path.append(_p)

import math
import numpy as np
import ml_dtypes

import concourse.bass as bass
import concourse.mybir as mybir
import concourse.tile as tile
from concourse import bacc, bass_utils
from concourse.masks import make_identity

F32 = mybir.dt.float32
F32R = mybir.dt.float32r
BF16 = mybir.dt.bfloat16
AFT = mybir.ActivationFunctionType

P = 128
D = 2048
HD = 64
NJ = HD // 2          # 32 rope freqs
OQ = 512              # q-head dims per core (8 heads * 64)
OKV = 128             # kv-head dims per core (2 heads * 64)
NPAIR = 4             # head pairs per core
DT = D // P           # 16 d-tiles

HEAD_PERM = [0, 4, 1, 5, 2, 6, 3, 7]


def _emit_rope(nc, out_sb, in_ap, cos_ap, sin_ap, nh, tmp_pool):
    """RoPE: out[.., 2j] = x0*c - x1*s ; out[.., 2j+1] = x0*s + x1*c.
    in_ap/out_sb: [128, nh*64]; cos_ap/sin_ap: [128, 32] (per s-tile)."""
    w = nh * NJ
    x = in_ap.rearrange("p (h j t) -> p h j t", h=nh, j=NJ, t=2)
    o = out_sb.rearrange("p (h j t) -> p h j t", h=nh, j=NJ, t=2)
    x0, x1 = x[:, :, :, 0], x[:, :, :, 1]
    o0, o1 = o[:, :, :, 0], o[:, :, :, 1]
    c = cos_ap.unsqueeze(1).broadcast_to([P, nh, NJ])
    s = sin_ap.unsqueeze(1).broadcast_to([P, nh, NJ])
    ta = tmp_pool.tile([P, w], F32, tag="rope_ta")
    tb = tmp_pool.tile([P, w], F32, tag="rope_tb")
    ta3 = ta.rearrange("p (h j) -> p h j", h=nh, j=NJ)
    tb3 = tb.rearrange("p (h j) -> p h j", h=nh, j=NJ)
    nc.vector.tensor_mul(ta3, x0, c)
    nc.vector.tensor_mul(tb3, x1, s)
    nc.vector.tensor_sub(o0, ta3, tb3)
    nc.vector.tensor_mul(ta3, x0, s)
    nc.vector.tensor_mul(tb3, x1, c)
    nc.vector.tensor_add(o1, ta3, tb3)


def emit_kernel(nc, tc, ctx, S):
    NSC = S // 512        # s-chunks
    NST = S // P          # s-tiles (global)

    # pre-transposed inputs (declared float32r: consumed raw by f32r matmuls)
    xT_d = nc.dram_tensor("xT", [D, S], BF16, kind="ExternalInput").ap()
    wqT_d = nc.dram_tensor("wqT", [D, OQ], BF16, kind="ExternalInput").ap()
    wkvT_d = nc.dram_tensor("wkvT", [D, 256], BF16, kind="ExternalInput").ap()
    woT_d = nc.dram_tensor("woT", [OQ, D], BF16, kind="ExternalInput").ap()
    cos_d = nc.dram_tensor("cost", [S, NJ], F32, kind="ExternalInput").ap()
    sin_d = nc.dram_tensor("sint", [S, NJ], F32, kind="ExternalInput").ap()
    out_d = nc.dram_tensor("out", [S, D], F32, kind="ExternalOutput").ap()

    ctx.enter_context(nc.allow_low_precision(reason="float32r tiles feed matmuls"))
    const = ctx.enter_context(tc.tile_pool(name="const", bufs=1))
    work = ctx.enter_context(tc.tile_pool(name="work", bufs=2))
    epool = ctx.enter_context(tc.tile_pool(name="epool", bufs=4))
    xTp = ctx.enter_context(tc.tile_pool(name="xTp", bufs=2))
    qTp = ctx.enter_context(tc.tile_pool(name="qTp", bufs=2))
    atp = ctx.enter_context(tc.tile_pool(name="atp", bufs=2))
    psA = ctx.enter_context(tc.tile_pool(name="psA", bufs=2, space="PSUM"))
    psB = ctx.enter_context(tc.tile_pool(name="psB", bufs=2, space="PSUM"))

    idn = const.tile([P, P], F32)
    make_identity(nc, idn)
    ones_f = const.tile([P, 1], F32)
    nc.any.memset(ones_f[:], 1.0)
    ones64 = const.tile([65, HD], F32R)
    nc.vector.tensor_copy(ones64[:], ones_f[0:65, 0:1].broadcast_to([65, HD]))

    wqT = const.tile([P, DT * OQ], BF16)    # [d_loc, dt*512 + o']
    wkvT = const.tile([P, DT * 256], BF16)  # [d_loc, dt*256 + (k:0-127 | v:128-255)]
    woT = const.tile([P, NPAIR * D], BF16)  # [o'_loc, p*2048 + d]
    kT = const.tile([P, S], BF16)           # [o_kv, s]
    v2 = const.tile([P, NST * 130], BF16)   # [s_loc, g*130 + a*65 + (hd|one)]
    cosr = const.tile([P, NST * NJ], F32)
    sinr = const.tile([P, NST * NJ], F32)

    # bulk weight loads (single DMAs, [row-tile -> partition] reshapes)
    nc.sync.dma_start(wqT[:].rearrange("p (dt o) -> p dt o", dt=DT, o=OQ),
                      wqT_d.rearrange("(dt p) o -> p dt o", p=P))
    nc.sync.dma_start(wkvT[:].rearrange("p (dt o) -> p dt o", dt=DT, o=256),
                      wkvT_d.rearrange("(dt p) o -> p dt o", p=P))
    nc.sync.dma_start(woT[:].rearrange("p (pp d) -> p pp d", pp=NPAIR, d=D),
                      woT_d.rearrange("(pp o) d -> o pp d", o=P))
    nc.sync.dma_start(cosr[:].rearrange("p (g j) -> p g j", g=NST, j=NJ),
                      cos_d.rearrange("(g p) j -> p g j", p=P))
    nc.sync.dma_start(sinr[:].rearrange("p (g j) -> p g j", g=NST, j=NJ),
                      sin_d.rearrange("(g p) j -> p g j", p=P))

    # ones columns of v2 (positions i*65 + 64)
    v2ones = v2[:].rearrange("p (i c) -> p i c", i=2 * NST, c=65)[:, :, 64]
    nc.vector.tensor_copy(v2ones, ones_f[:, 0:1].broadcast_to([P, 2 * NST]))

    def transpose_block(src_ap, dst_ap):
        tp = psA.tile([P, P], F32, tag="tp", bufs=1)
        nc.tensor.transpose(tp[:], src_ap, idn[:])
        nc.vector.tensor_copy(dst_ap, tp[:])

    # ---- main loop over s-chunks ----
    for c in range(NSC):
        qT = qTp.tile([P, NPAIR * 512], BF16, tag="qT")  # [o'_loc, p*512 + q_loc]
        attnT = atp.tile([P, NPAIR * 512], BF16, tag="attnT")

        for st in range(4):
            g = c * 4 + st
            xT = xTp.tile([P, DT * P], BF16, tag="xT")  # [d_loc, dt*128 + s_loc]
            nc.sync.dma_start(xT[:].rearrange("p (dt s) -> p dt s", dt=DT, s=P),
                              xT_d[:, g * P:(g + 1) * P].rearrange("(dt p) s -> p dt s", p=P))
            cos_ap = cosr[:, g * NJ:(g + 1) * NJ]
            sin_ap = sinr[:, g * NJ:(g + 1) * NJ]
            # q projection [s,o'] and rope
            qp = psA.tile([P, OQ], F32, tag="proj")
            for dt in range(DT):
                nc.tensor.matmul(qp[:], xT[:, dt * P:(dt + 1) * P],
                                 wqT[:, dt * OQ:(dt + 1) * OQ],
                                 start=(dt == 0), stop=(dt == DT - 1))
            qr = work.tile([P, OQ], F32, tag="qr")
            _emit_rope(nc, qr[:], qp[:], cos_ap, sin_ap, 8, work)
            # kv projection [s, k(128)|v(128)] and rope on k part
            kvp = psA.tile([P, 256], F32, tag="proj")
            for dt in range(DT):
                nc.tensor.matmul(kvp[:], xT[:, dt * P:(dt + 1) * P],
                                 wkvT[:, dt * 256:(dt + 1) * 256],
                                 start=(dt == 0), stop=(dt == DT - 1))
            kr = work.tile([P, OKV], F32, tag="kr")
            _emit_rope(nc, kr[:], kvp[:, 0:OKV], cos_ap, sin_ap, 2, work)
            # v -> v2 (split the two kv heads around the ones columns)
            v_src = kvp[:, OKV:256].rearrange("p (a x) -> p a x", a=2, x=HD)
            v_dst = v2[:, g * 130:(g + 1) * 130].rearrange("p (a x) -> p a x", a=2, x=65)[:, :, 0:HD]
            nc.vector.tensor_copy(v_dst, v_src)
            # transposes q -> qT, k -> kT
            for p in range(NPAIR):
                transpose_block(qr[:, p * P:(p + 1) * P],
                                qT[:, p * 512 + st * P: p * 512 + (st + 1) * P])
            transpose_block(kr[:], kT[:, g * P:(g + 1) * P])

        # ---- attention for q-chunk c ----
        NJT = 4 * (c + 1)
        for p in range(NPAIR):
            pva = psB.tile([65, 512], F32, tag="pv")
            pvb = psB.tile([65, 512], F32, tag="pv")
            for j in range(NJT):
                sca = psA.tile([P, 512], F32, tag="sc", bufs=3)
                scb = psA.tile([P, 512], F32, tag="sc", bufs=3)
                nc.tensor.matmul(sca[:], kT[0:HD, j * P:(j + 1) * P],
                                 qT[0:HD, p * 512:(p + 1) * 512])
                nc.tensor.matmul(scb[:], kT[HD:P, j * P:(j + 1) * P],
                                 qT[HD:P, p * 512:(p + 1) * 512])
                ea = epool.tile([P, 512], BF16, tag="e", bufs=6)
                eb = epool.tile([P, 512], BF16, tag="e", bufs=6)
                nc.scalar.activation(ea[:], sca[:], AFT.Exp, scale=1.0 / 8.0)
                nc.scalar.activation(eb[:], scb[:], AFT.Exp, scale=1.0 / 8.0)
                if j >= 4 * c:  # diagonal block: zero where k_glob > q_glob
                    for e_t in (ea, eb):
                        nc.gpsimd.affine_select(
                            out=e_t[:], in_=e_t[:],
                            compare_op=mybir.AluOpType.is_ge, fill=0.0,
                            base=c * 512 - j * P, channel_multiplier=-1,
                            pattern=[[1, 512]])
                nc.tensor.matmul(pva[:], v2[:, j * 130: j * 130 + 65], ea[:],
                                 start=(j == 0), stop=(j == NJT - 1), skip_group_check=True)
                nc.tensor.matmul(pvb[:], v2[:, j * 130 + 65: (j + 1) * 130], eb[:],
                                 start=(j == 0), stop=(j == NJT - 1), skip_group_check=True)
            # normalize: attnT rows = outT * Zinv ; Z sits in psum row 64
            for half, pv in enumerate((pva, pvb)):
                # evacuate PSUM immediately so the pv slot frees for the next pair
                pvs = work.tile([65, 512], F32, tag="pvs", bufs=4)
                nc.scalar.copy(pvs[:], pv[:])
                zf = work.tile([65, 512], F32, tag="lnz")
                zi = work.tile([65, 512], F32R, tag="rc")
                nc.vector.reciprocal(zf[64:65, :], pvs[64:65, :])
                nc.vector.tensor_copy(zi[64:65, :], zf[64:65, :])
                bc = psA.tile([HD, 512], F32, tag="sc", bufs=3)
                nc.tensor.matmul(bc[:], ones64[64:65, :], zi[64:65, :])
                bcs = work.tile([HD, 512], F32, tag="bc")
                nc.scalar.copy(bcs[:], bc[:])
                if half == 0:
                    nc.vector.tensor_mul(attnT[0:HD, p * 512:(p + 1) * 512],
                                         pvs[0:HD, :], bcs[:])
                else:
                    tmpb = work.tile([HD, 512], BF16, tag="tmpb")
                    nc.vector.tensor_mul(tmpb[:], pvs[0:HD, :], bcs[:])
                    # partition shift 0:64 -> 64:128 via sbuf-sbuf DMA
                    nc.sync.dma_start(attnT[HD:P, p * 512:(p + 1) * 512], tmpb[:])

        # ---- final: res[s, d] = sum_p attnT_p.T @ woT_p ----
        for st in range(4):
            for dc in range(4):
                rp = psB.tile([P, 512], F32, tag="pv")
                for p in range(NPAIR):
                    nc.tensor.matmul(rp[:], attnT[:, p * 512 + st * P: p * 512 + (st + 1) * P],
                                     woT[:, p * D + dc * 512: p * D + (dc + 1) * 512],
                                     start=(p == 0), stop=(p == NPAIR - 1), skip_group_check=True)
                rs = work.tile([P, 512], F32, tag="rs")
                nc.scalar.copy(rs[:], rp[:])
                nc.sync.dma_start(out_d[(c * 4 + st) * P:(c * 4 + st + 1) * P,
                                        dc * 512:(dc + 1) * 512], rs[:])


_NC_CACHE = {}


def build(S=2048):
    if S in _NC_CACHE:
        return _NC_CACHE[S]
    from contextlib import ExitStack
    nc = bacc.Bacc("TRN2", target_bir_lowering=False, debug=False, num_devices=8)
    with tile.TileContext(nc) as tc, ExitStack() as ctx:
        emit_kernel(nc, tc, ctx, S)
    nc.compile()
    _NC_CACHE[S] = nc
    return nc


def shard_inputs(x, theta, wq, wk, wv, wo, S=2048):
    """Returns in_maps for 8 cores: core = b*4 + g. Pure layout prep."""
    cost = np.cos(theta[:S]).astype(np.float32)
    sint = np.sin(theta[:S]).astype(np.float32)
    in_maps = []
    for core in range(8):
        b, g = core // 4, core % 4
        wq_g = wq[g * 512:(g + 1) * 512].reshape(8, HD, D)[HEAD_PERM].reshape(512, D)
        wo_g = wo[:, g * 512:(g + 1) * 512].reshape(D, 8, HD)[:, HEAD_PERM].reshape(D, 512)
        wkv_g = np.concatenate([wk[g * 128:(g + 1) * 128], wv[g * 128:(g + 1) * 128]], axis=0)
        bf = ml_dtypes.bfloat16
        in_maps.append({
            "xT": np.ascontiguousarray(x[b, :S].T).astype(bf),
            "wqT": np.ascontiguousarray(wq_g.T).astype(bf),
            "wkvT": np.ascontiguousarray(wkv_g.T).astype(bf),
            "woT": np.ascontiguousarray(wo_g.T).astype(bf),
            "cost": cost,
            "sint": sint,
        })
    return in_maps


def run_on_hw(inputs, S=2048, trace=False):
    nc = build(S)
    in_maps = shard_inputs(inputs["x"], inputs["theta"], inputs["wq"],
                           inputs["wk"], inputs["wv"], inputs["wo"], S=S)
    res = bass_utils.run_bass_kernel_spmd(nc, in_maps, core_ids=list(range(8)),
                                          trace=trace)
    parts = [res.results[c]["out"] for c in range(8)]
    out = np.stack([parts[0] + parts[1] + parts[2] + parts[3],
                    parts[4] + parts[5] + parts[6] + parts[7]], axis=0)
    return out, res


def kernel(x, theta, mask, wq, wk, wv, wo):
    out, _ = run_on_hw({"x": np.asarray(x, np.float32), "theta": np.asarray(theta, np.float32),
                        "wq": np.asarray(wq, np.float32), "wk": np.asarray(wk, np.float32),
                        "wv": np.asarray(wv, np.float32), "wo": np.asarray(wo, np.float32)})
    return out


# revision 18
# speedup vs baseline: 1.3015x; 1.3015x over previous
"""Trainium2 Bass kernel for GQA attention (B=2, S=2048, D=2048, H=32, KVH=8).

Sharding: 8 cores = 2 batches x 4 head-groups. Each core handles one batch and
8 q-heads / 2 kv-heads: wq/wk/wv column-parallel, wo row-parallel; the partial
wo products are summed on the host.

Host-side prep (pure layout, no math): inputs are sharded, head-permuted and
pre-transposed so every matmul operand DMAs straight into its [K-on-partition]
layout; cos/sin of the rope angles are also computed host-side (the ScalarE Sin
LUT only covers [-pi, pi]).

Per-core kernel (all matmuls float32r):
  - q/k/v projections computed with s on partitions ([s,o] layout) from the
    pre-transposed xT/wqT/wkvT, RoPE applied with strided DVE ops, then q/k
    transposed on PE to [o,s] for attention.
  - scores are computed transposed: scT[k,q] = kT.T @ qT per head; exp on ACT;
    causal handled by skipping fully-masked k-tiles + affine_select on diagonal.
  - PV: lhsT = [v_head | ones] (M=65) so the softmax denominator accumulates in
    PSUM row 64 alongside the output.  attnT = outT * Zinv with Zinv = exp(-ln Z)
    (two small ACT ops) broadcast across partitions via a K=1 PE matmul.
  - final: res[s,d] = sum_p attnT_p.T @ woT_p, accumulated over 4 o-blocks.

Head order within a core is permuted to [0,4,1,5,2,6,3,7] so that each
128-partition block pairs head h (kv0) with h+4 (kv1), letting the K=64 score
matmuls row-pack two heads concurrently on the PE array.
"""

import os
import sys

for _p in ("/opt/trn_rl_repo", "/root/.axon_site/_ro/trn_rl_repo"):
    if os.path.isdir(_p) and _p not in sys.path:
        sys.path.append(_p)

import math
import numpy as np
import ml_dtypes

import concourse.bass as bass
import concourse.mybir as mybir
import concourse.tile as tile
from concourse import bacc, bass_utils
from concourse.masks import make_identity

F32 = mybir.dt.float32
F32R = mybir.dt.float32r
BF16 = mybir.dt.bfloat16
AFT = mybir.ActivationFunctionType

P = 128
D = 2048
HD = 64
NJ = HD // 2          # 32 rope freqs
OQ = 512              # q-head dims per core (8 heads * 64)
OKV = 128             # kv-head dims per core (2 heads * 64)
NPAIR = 4             # head pairs per core
DT = D // P           # 16 d-tiles

HEAD_PERM = [0, 4, 1, 5, 2, 6, 3, 7]


def _emit_rope(nc, out_sb, in_ap, cos_ap, sin_ap, nh, tmp_pool):
    """RoPE: out[.., 2j] = x0*c - x1*s ; out[.., 2j+1] = x0*s + x1*c.
    in_ap/out_sb: [128, nh*64]; cos_ap/sin_ap: [128, 32] (per s-tile)."""
    w = nh * NJ
    x = in_ap.rearrange("p (h j t) -> p h j t", h=nh, j=NJ, t=2)
    o = out_sb.rearrange("p (h j t) -> p h j t", h=nh, j=NJ, t=2)
    x0, x1 = x[:, :, :, 0], x[:, :, :, 1]
    o0, o1 = o[:, :, :, 0], o[:, :, :, 1]
    c = cos_ap.unsqueeze(1).broadcast_to([P, nh, NJ])
    s = sin_ap.unsqueeze(1).broadcast_to([P, nh, NJ])
    ta = tmp_pool.tile([P, w], F32, tag="rope_ta")
    tb = tmp_pool.tile([P, w], F32, tag="rope_tb")
    ta3 = ta.rearrange("p (h j) -> p h j", h=nh, j=NJ)
    tb3 = tb.rearrange("p (h j) -> p h j", h=nh, j=NJ)
    nc.vector.tensor_mul(ta3, x0, c)
    nc.vector.tensor_mul(tb3, x1, s)
    nc.vector.tensor_sub(o0, ta3, tb3)
    nc.vector.tensor_mul(ta3, x0, s)
    nc.vector.tensor_mul(tb3, x1, c)
    nc.vector.tensor_add(o1, ta3, tb3)


def emit_kernel(nc, tc, ctx, S):
    NSC = S // 512        # s-chunks
    NST = S // P          # s-tiles (global)

    # pre-transposed inputs (declared float32r: consumed raw by f32r matmuls)
    xT_d = nc.dram_tensor("xT", [D, S], BF16, kind="ExternalInput").ap()
    wqT_d = nc.dram_tensor("wqT", [D, OQ], BF16, kind="ExternalInput").ap()
    wkvT_d = nc.dram_tensor("wkvT", [D, 256], BF16, kind="ExternalInput").ap()
    woT_d = nc.dram_tensor("woT", [OQ, D], BF16, kind="ExternalInput").ap()
    cos_d = nc.dram_tensor("cost", [S, NJ], F32, kind="ExternalInput").ap()
    sin_d = nc.dram_tensor("sint", [S, NJ], F32, kind="ExternalInput").ap()
    out_d = nc.dram_tensor("out", [S, D], F32, kind="ExternalOutput").ap()

    ctx.enter_context(nc.allow_low_precision(reason="float32r tiles feed matmuls"))
    const = ctx.enter_context(tc.tile_pool(name="const", bufs=1))
    work = ctx.enter_context(tc.tile_pool(name="work", bufs=2))
    epool = ctx.enter_context(tc.tile_pool(name="epool", bufs=4))
    xTp = ctx.enter_context(tc.tile_pool(name="xTp", bufs=2))
    qTp = ctx.enter_context(tc.tile_pool(name="qTp", bufs=2))
    atp = ctx.enter_context(tc.tile_pool(name="atp", bufs=2))
    psA = ctx.enter_context(tc.tile_pool(name="psA", bufs=2, space="PSUM"))
    psB = ctx.enter_context(tc.tile_pool(name="psB", bufs=2, space="PSUM"))

    idn = const.tile([P, P], F32)
    make_identity(nc, idn)
    ones_f = const.tile([P, 1], F32)
    nc.any.memset(ones_f[:], 1.0)
    ones64 = const.tile([65, HD], F32R)
    nc.vector.tensor_copy(ones64[:], ones_f[0:65, 0:1].broadcast_to([65, HD]))

    wqT = const.tile([P, DT * OQ], BF16)    # [d_loc, dt*512 + o']
    wkvT = const.tile([P, DT * 256], BF16)  # [d_loc, dt*256 + (k:0-127 | v:128-255)]
    woT = const.tile([P, NPAIR * D], BF16)  # [o'_loc, p*2048 + d]
    kT = const.tile([P, S], BF16)           # [o_kv, s]
    v2 = const.tile([P, NST * 130], BF16)   # [s_loc, g*130 + a*65 + (hd|one)]
    cosr = const.tile([P, NST * NJ], F32)
    sinr = const.tile([P, NST * NJ], F32)

    # bulk weight loads (single DMAs, [row-tile -> partition] reshapes)
    nc.sync.dma_start(wqT[:].rearrange("p (dt o) -> p dt o", dt=DT, o=OQ),
                      wqT_d.rearrange("(dt p) o -> p dt o", p=P))
    nc.sync.dma_start(wkvT[:].rearrange("p (dt o) -> p dt o", dt=DT, o=256),
                      wkvT_d.rearrange("(dt p) o -> p dt o", p=P))
    nc.sync.dma_start(woT[:].rearrange("p (pp d) -> p pp d", pp=NPAIR, d=D),
                      woT_d.rearrange("(pp o) d -> o pp d", o=P))
    nc.sync.dma_start(cosr[:].rearrange("p (g j) -> p g j", g=NST, j=NJ),
                      cos_d.rearrange("(g p) j -> p g j", p=P))
    nc.sync.dma_start(sinr[:].rearrange("p (g j) -> p g j", g=NST, j=NJ),
                      sin_d.rearrange("(g p) j -> p g j", p=P))

    # ones columns of v2 (positions i*65 + 64)
    v2ones = v2[:].rearrange("p (i c) -> p i c", i=2 * NST, c=65)[:, :, 64]
    nc.vector.tensor_copy(v2ones, ones_f[:, 0:1].broadcast_to([P, 2 * NST]))

    def transpose_block(src_ap, dst_ap):
        tp = psA.tile([P, P], F32, tag="sc", bufs=2)
        nc.tensor.transpose(tp[:], src_ap, idn[:])
        nc.vector.tensor_copy(dst_ap, tp[:])

    # ---- main loop over s-chunks ----
    for c in range(NSC):
        qT = qTp.tile([P, NPAIR * 512], BF16, tag="qT")  # [o'_loc, p*512 + q_loc]
        attnT = atp.tile([P, NPAIR * 512], BF16, tag="attnT")

        for st in range(4):
            g = c * 4 + st
            xT = xTp.tile([P, DT * P], BF16, tag="xT")  # [d_loc, dt*128 + s_loc]
            nc.sync.dma_start(xT[:].rearrange("p (dt s) -> p dt s", dt=DT, s=P),
                              xT_d[:, g * P:(g + 1) * P].rearrange("(dt p) s -> p dt s", p=P))
            cos_ap = cosr[:, g * NJ:(g + 1) * NJ]
            sin_ap = sinr[:, g * NJ:(g + 1) * NJ]
            # q projection [s,o'] and rope
            qp = psA.tile([P, OQ], F32, tag="sc", bufs=2)
            for dt in range(DT):
                nc.tensor.matmul(qp[:], xT[:, dt * P:(dt + 1) * P],
                                 wqT[:, dt * OQ:(dt + 1) * OQ],
                                 start=(dt == 0), stop=(dt == DT - 1))
            qr = work.tile([P, OQ], F32, tag="qr")
            _emit_rope(nc, qr[:], qp[:], cos_ap, sin_ap, 8, work)
            # kv projection [s, k(128)|v(128)] and rope on k part
            kvp = psA.tile([P, 256], F32, tag="sc", bufs=2)
            for dt in range(DT):
                nc.tensor.matmul(kvp[:], xT[:, dt * P:(dt + 1) * P],
                                 wkvT[:, dt * 256:(dt + 1) * 256],
                                 start=(dt == 0), stop=(dt == DT - 1))
            kr = work.tile([P, OKV], F32, tag="kr")
            _emit_rope(nc, kr[:], kvp[:, 0:OKV], cos_ap, sin_ap, 2, work)
            # v -> v2 (split the two kv heads around the ones columns)
            v_src = kvp[:, OKV:256].rearrange("p (a x) -> p a x", a=2, x=HD)
            v_dst = v2[:, g * 130:(g + 1) * 130].rearrange("p (a x) -> p a x", a=2, x=65)[:, :, 0:HD]
            nc.vector.tensor_copy(v_dst, v_src)
            # transposes q -> qT, k -> kT
            for p in range(NPAIR):
                transpose_block(qr[:, p * P:(p + 1) * P],
                                qT[:, p * 512 + st * P: p * 512 + (st + 1) * P])
            transpose_block(kr[:], kT[:, g * P:(g + 1) * P])

        # ---- attention for q-chunk c ----
        NJT = 4 * (c + 1)
        for pg in range(2):          # two pair-groups, 2 head-pairs each, interleaved
            pvt = {}
            for pp in range(2):
                for half in range(2):
                    pvt[(pp, half)] = psB.tile([65, 512], F32, tag="pv", bufs=4,
                                               name=f"pv_{c}_{pg}_{pp}_{half}")
            for j in range(NJT):
                for pp in range(2):
                    p = pg * 2 + pp
                    sc2 = psA.tile([P, 1024], F32, tag="sc", bufs=2)
                    nc.tensor.matmul(sc2[:, 0:512], kT[0:HD, j * P:(j + 1) * P],
                                     qT[0:HD, p * 512:(p + 1) * 512])
                    nc.tensor.matmul(sc2[:, 512:1024], kT[HD:P, j * P:(j + 1) * P],
                                     qT[HD:P, p * 512:(p + 1) * 512])
                    e2 = epool.tile([P, 1024], BF16, tag="e", bufs=4)
                    nc.scalar.activation(e2[:], sc2[:], AFT.Exp, scale=1.0 / 8.0)
                    if j >= 4 * c:  # diagonal block: zero where k_glob > q_glob
                        for half in range(2):
                            nc.gpsimd.affine_select(
                                out=e2[:, half * 512:(half + 1) * 512],
                                in_=e2[:, half * 512:(half + 1) * 512],
                                compare_op=mybir.AluOpType.is_ge, fill=0.0,
                                base=c * 512 - j * P, channel_multiplier=-1,
                                pattern=[[1, 512]])
                    nc.tensor.matmul(pvt[(pp, 0)], v2[:, j * 130: j * 130 + 65],
                                     e2[:, 0:512],
                                     start=(j == 0), stop=(j == NJT - 1), skip_group_check=True)
                    nc.tensor.matmul(pvt[(pp, 1)], v2[:, j * 130 + 65: (j + 1) * 130],
                                     e2[:, 512:1024],
                                     start=(j == 0), stop=(j == NJT - 1), skip_group_check=True)
            # normalize: attnT rows = outT * Zinv ; Z sits in psum row 64
            for pp in range(2):
                p = pg * 2 + pp
                for half in range(2):
                    pv = pvt[(pp, half)]
                    zf = work.tile([65, 512], F32, tag="lnz")
                    zi = work.tile([65, 512], F32R, tag="rc")
                    nc.vector.reciprocal(zf[64:65, :], pv[64:65, :])
                    nc.vector.tensor_copy(zi[64:65, :], zf[64:65, :])
                    bc = psA.tile([HD, 512], F32, tag="sc", bufs=2)
                    nc.tensor.matmul(bc[:], ones64[64:65, :], zi[64:65, :])
                    bcs = work.tile([HD, 512], F32, tag="bc")
                    nc.vector.tensor_copy(bcs[:], bc[:])
                    if half == 0:
                        nc.vector.tensor_mul(attnT[0:HD, p * 512:(p + 1) * 512],
                                             pv[0:HD, :], bcs[:])
                    else:
                        tmpb = work.tile([HD, 512], BF16, tag="tmpb")
                        nc.vector.tensor_mul(tmpb[:], pv[0:HD, :], bcs[:])
                        # partition shift 0:64 -> 64:128 via sbuf-sbuf DMA
                        nc.sync.dma_start(attnT[HD:P, p * 512:(p + 1) * 512], tmpb[:])

        # ---- final: res[s, d] = sum_p attnT_p.T @ woT_p ----
        for st in range(4):
            for dc in range(4):
                rp = psB.tile([P, 512], F32, tag="pv", bufs=4)
                for p in range(NPAIR):
                    nc.tensor.matmul(rp[:], attnT[:, p * 512 + st * P: p * 512 + (st + 1) * P],
                                     woT[:, p * D + dc * 512: p * D + (dc + 1) * 512],
                                     start=(p == 0), stop=(p == NPAIR - 1), skip_group_check=True)
                rs = work.tile([P, 512], F32, tag="rs")
                nc.scalar.copy(rs[:], rp[:])
                nc.sync.dma_start(out_d[(c * 4 + st) * P:(c * 4 + st + 1) * P,
                                        dc * 512:(dc + 1) * 512], rs[:])


_NC_CACHE = {}


def build(S=2048):
    if S in _NC_CACHE:
        return _NC_CACHE[S]
    from contextlib import ExitStack
    nc = bacc.Bacc("TRN2", target_bir_lowering=False, debug=False, num_devices=8)
    with tile.TileContext(nc) as tc, ExitStack() as ctx:
        emit_kernel(nc, tc, ctx, S)
    nc.compile()
    _NC_CACHE[S] = nc
    return nc


def shard_inputs(x, theta, wq, wk, wv, wo, S=2048):
    """Returns in_maps for 8 cores: core = b*4 + g. Pure layout prep."""
    cost = np.cos(theta[:S]).astype(np.float32)
    sint = np.sin(theta[:S]).astype(np.float32)
    in_maps = []
    for core in range(8):
        b, g = core // 4, core % 4
        wq_g = wq[g * 512:(g + 1) * 512].reshape(8, HD, D)[HEAD_PERM].reshape(512, D)
        wo_g = wo[:, g * 512:(g + 1) * 512].reshape(D, 8, HD)[:, HEAD_PERM].reshape(D, 512)
        wkv_g = np.concatenate([wk[g * 128:(g + 1) * 128], wv[g * 128:(g + 1) * 128]], axis=0)
        bf = ml_dtypes.bfloat16
        in_maps.append({
            "xT": np.ascontiguousarray(x[b, :S].T).astype(bf),
            "wqT": np.ascontiguousarray(wq_g.T).astype(bf),
            "wkvT": np.ascontiguousarray(wkv_g.T).astype(bf),
            "woT": np.ascontiguousarray(wo_g.T).astype(bf),
            "cost": cost,
            "sint": sint,
        })
    return in_maps


def run_on_hw(inputs, S=2048, trace=False):
    nc = build(S)
    in_maps = shard_inputs(inputs["x"], inputs["theta"], inputs["wq"],
                           inputs["wk"], inputs["wv"], inputs["wo"], S=S)
    res = bass_utils.run_bass_kernel_spmd(nc, in_maps, core_ids=list(range(8)),
                                          trace=trace)
    parts = [res.results[c]["out"] for c in range(8)]
    out = np.stack([parts[0] + parts[1] + parts[2] + parts[3],
                    parts[4] + parts[5] + parts[6] + parts[7]], axis=0)
    return out, res


def kernel(x, theta, mask, wq, wk, wv, wo):
    out, _ = run_on_hw({"x": np.asarray(x, np.float32), "theta": np.asarray(theta, np.float32),
                        "wq": np.asarray(wq, np.float32), "wk": np.asarray(wk, np.float32),
                        "wv": np.asarray(wv, np.float32), "wo": np.asarray(wo, np.float32)})
    return out
